# revision 1
# baseline (speedup 1.0000x reference)
"""Trainium2 Bass kernel for additive (Bahdanau-style) attention.

Reference computation (per batch b):
    w1 = matrix @ W1_w + W1_b                  # [N, A]
    w2 = matrix @ W2_w + W2_b                  # [N, A]
    scores[i, j] = v . tanh(w1[i] + w2[j])     # [N, N]
    attn = softmax(where(mask, scores, -inf))  # [N, N]
    out = attn @ matrix                        # [N, D]

Shapes: B=4, N=512, D=768, A=128.

Sharding: 8 cores = (batch b = core//2) x (query half = core%2). Each core
owns 256 queries of one batch; all compute is core-local (no collectives).

Per-core dataflow (all "transposed" so no on-chip transposes are needed):
  - w1T [A=128part, 256q], w2T [A, 512k] via PE matmuls from matrix^T.
  - per query q: DVE tensor_scalar broadcast-add  sums = w2T + w1T[:, q]
    (fp32 2x mode), ScalarE tanh on groups of 8 queries (bf16 out).
  - scores^T column: PE matmul lhsT=tanh chunk [A,128k] (bf16, FWL),
    rhs=v [A,1] -> psum column [128k, 1]; 4 key chunks -> S^T in PSUM
    packed [128kpart, (kc, q) free].
  - softmax without max-subtraction (scores are bounded ~|s|<=9):
    exp on ScalarE (PSUM->SBUF, bf16), mask multiply on DVE,
    row sums via PE matmul with an appended ones-column on the AV rhs.
  - AV: out[q, d] = sum_k P^T[k, q] * V[k, d]: lhsT = P^T chunk, rhs = V
    (bf16), accumulated over 4 key chunks; normalization by 1/rowsum fused
    into the PSUM->SBUF copy (per-partition tensor_scalar).
"""

import numpy as np

_B, _N, _D, _A = 4, 512, 768, 128
_NC = 8
_QPC = (_B * _N) // _NC  # 256 queries per core
_P = 128
_KD = _D // _P  # 6 contraction chunks over D
_KC = _N // _P  # 4 key chunks
_QG = 8         # queries per tanh group (ScalarE call)

_CACHE = {}


def _build_nc():
    import concourse.tile as tile
    from concourse import bacc, mybir

    f32 = mybir.dt.float32
    bf16 = mybir.dt.bfloat16
    i32 = mybir.dt.int32

    nc = bacc.Bacc(
        "TRN2",
        target_bir_lowering=False,
        debug=False,
        num_devices=1,
    )

    # Per-core inputs (host does only slicing / transposition).
    matT = nc.dram_tensor("matT", [_D, _N], f32, kind="ExternalInput").ap()
    matTq = nc.dram_tensor("matTq", [_D, _QPC], f32, kind="ExternalInput").ap()
    matv = nc.dram_tensor("matv", [_N, _D], f32, kind="ExternalInput").ap()
    maskT = nc.dram_tensor("maskT", [_N, _QPC], i32, kind="ExternalInput").ap()
    w1w = nc.dram_tensor("w1w", [_D, _A], f32, kind="ExternalInput").ap()
    w2w = nc.dram_tensor("w2w", [_D, _A], f32, kind="ExternalInput").ap()
    w1b = nc.dram_tensor("w1b", [_A, 1], f32, kind="ExternalInput").ap()
    w2b = nc.dram_tensor("w2b", [_A, 1], f32, kind="ExternalInput").ap()
    vw = nc.dram_tensor("vw", [_A, 1], f32, kind="ExternalInput").ap()
    out = nc.dram_tensor("out", [_QPC, _D], f32, kind="ExternalOutput").ap()

    with tile.TileContext(nc) as tc:
        _kernel_body(
            tc, mybir, matT, matTq, matv, maskT, w1w, w2w, w1b, w2b, vw, out
        )
    nc.compile()
    return nc


def _kernel_body(tc, mybir, matT, matTq, matv, maskT, w1w, w2w, w1b, w2b, vw, out):
    nc = tc.nc
    f32 = mybir.dt.float32
    bf16 = mybir.dt.bfloat16
    i32 = mybir.dt.int32
    Tanh = mybir.ActivationFunctionType.Tanh
    Exp = mybir.ActivationFunctionType.Exp
    P, N, D, A, QPC = _P, _N, _D, _A, _QPC
    KD, KC, QG = _KD, _KC, _QG
    NG = P // QG  # tanh groups per 128-query block

    with (
        tc.tile_pool(name="const", bufs=1) as const,
        tc.tile_pool(name="sums", bufs=2) as sums_pool,
        tc.tile_pool(name="tanh", bufs=2) as tanh_pool,
        tc.tile_pool(name="pt", bufs=2) as pt_pool,
        tc.tile_pool(name="osb", bufs=2) as osb_pool,
        tc.tile_pool(name="small", bufs=2) as small_pool,
        tc.tile_pool(name="psS", bufs=2, space="PSUM") as psS_pool,
        tc.tile_pool(name="psO1", bufs=2, space="PSUM") as psO1_pool,
        tc.tile_pool(name="psO2", bufs=2, space="PSUM") as psO2_pool,
    ):
        # ---------------- constants / inputs to SBUF ----------------
        matT_sb = const.tile([P, KD, N], f32)
        nc.sync.dma_start(matT_sb[:], matT.rearrange("(o p) n -> p o n", p=P))
        matTq_sb = const.tile([P, KD, QPC], f32)
        nc.sync.dma_start(matTq_sb[:], matTq.rearrange("(o p) n -> p o n", p=P))
        matv_sb = const.tile([P, KC, D], f32)
        nc.sync.dma_start(matv_sb[:], matv.rearrange("(o p) d -> p o d", p=P))
        mask_sb = const.tile([P, KC, QPC], i32)
        nc.sync.dma_start(mask_sb[:], maskT.rearrange("(o p) q -> p o q", p=P))
        w1w_sb = const.tile([P, KD, A], f32)
        nc.sync.dma_start(w1w_sb[:], w1w.rearrange("(o p) a -> p o a", p=P))
        w2w_sb = const.tile([P, KD, A], f32)
        nc.sync.dma_start(w2w_sb[:], w2w.rearrange("(o p) a -> p o a", p=P))
        w1b_sb = const.tile([P, 1], f32)
        nc.sync.dma_start(w1b_sb[:], w1b)
        w2b_sb = const.tile([P, 1], f32)
        nc.sync.dma_start(w2b_sb[:], w2b)
        vw_sb = const.tile([P, 1], f32)
        nc.sync.dma_start(vw_sb[:], vw)

        # bf16 casts (TensorE bf16 path needs both operands non-fp32)
        v_bf = const.tile([P, 1], bf16)
        nc.vector.tensor_copy(v_bf[:], vw_sb[:])
        mask_bf = const.tile([P, KC, QPC], bf16)
        nc.vector.tensor_copy(mask_bf[:], mask_sb[:])
        # AV rhs with an appended ones column (gives row-sums for free):
        # [:, kc, 0:768] = V chunk, [:, kc, 768] = 1.0
        mov_bf = const.tile([P, KC, D + 2], bf16)
        nc.vector.tensor_copy(mov_bf[:, :, 0:D], matv_sb[:])
        nc.vector.memset(mov_bf[:, :, D : D + 2], 1.0)

        # ---------------- projections: w1T [A, QPC], w2T [A, N] ----------------
        ps_w2 = psS_pool.tile([P, N], f32, tag="psS")
        for kd in range(KD):
            nc.tensor.matmul(
                ps_w2[:],
                lhsT=w2w_sb[:, kd, :],
                rhs=matT_sb[:, kd, :],
                start=(kd == 0),
                stop=(kd == KD - 1),
            )
        w2T_sb = const.tile([P, N], f32)
        nc.vector.tensor_scalar_add(w2T_sb[:], ps_w2[:], w2b_sb[:])

        ps_w1 = psS_pool.tile([P, N], f32, tag="psS")
        for kd in range(KD):
            nc.tensor.matmul(
                ps_w1[:, :QPC],
                lhsT=w1w_sb[:, kd, :],
                rhs=matTq_sb[:, kd, :],
                start=(kd == 0),
                stop=(kd == KD - 1),
            )
        w1T_sb = const.tile([P, QPC], f32)
        nc.vector.tensor_scalar_add(w1T_sb[:], ps_w1[:, :QPC], w1b_sb[:])

        # ---------------- main loop over 128-query blocks ----------------
        for qb in range(QPC // P):
            # scores^T for this block, packed [128 key-part, (kc, q) free]
            psS = psS_pool.tile([P, N], f32, tag="psS")
            for g in range(NG):
                sums = sums_pool.tile([P, QG, N], f32)
                for j in range(QG):
                    q = qb * P + g * QG + j
                    nc.vector.tensor_scalar_add(
                        sums[:, j, :], w2T_sb[:], w1T_sb[:, q : q + 1]
                    )
                th = tanh_pool.tile([P, QG, N], bf16)
                nc.scalar.activation(th[:], sums[:], Tanh)
                for j in range(QG):
                    ql = g * QG + j  # query index within block (0..127)
                    for kc in range(KC):
                        nc.tensor.matmul(
                            psS[:, kc * P + ql : kc * P + ql + 1],
                            lhsT=th[:, j, kc * P : (kc + 1) * P],
                            rhs=v_bf[:],
                            start=True,
                            stop=True,
                        )

            # exp (no max subtraction needed: |scores| <= sum|v| ~ 9)
            pt = pt_pool.tile([P, N], bf16)
            nc.scalar.activation(pt[:], psS[:], Exp)
            # mask: P^T *= mask^T  (bf16 2x)
            for kc in range(KC):
                nc.vector.tensor_mul(
                    pt[:, kc * P : (kc + 1) * P],
                    pt[:, kc * P : (kc + 1) * P],
                    mask_bf[:, kc, qb * P : (qb + 1) * P],
                )

            # AV + rowsum: out[q, d] = sum_kc P^T[kc].T @ [V | 1]
            psO1 = psO1_pool.tile([P, 512], f32, tag="psO1")
            psO2 = psO2_pool.tile([P, D - 512 + 2], f32, tag="psO2")
            for kc in range(KC):
                lhsT = pt[:, kc * P : (kc + 1) * P]
                nc.tensor.matmul(
                    psO1[:],
                    lhsT=lhsT,
                    rhs=mov_bf[:, kc, 0:512],
                    start=(kc == 0),
                    stop=(kc == KC - 1),
                )
                nc.tensor.matmul(
                    psO2[:],
                    lhsT=lhsT,
                    rhs=mov_bf[:, kc, 512 : D + 2],
                    start=(kc == 0),
                    stop=(kc == KC - 1),
                )

            recip = small_pool.tile([P, 1], f32)
            nc.vector.reciprocal(recip[:], psO2[:, D - 512 : D - 512 + 1])

            osb = osb_pool.tile([P, D], f32)
            nc.vector.tensor_scalar_mul(osb[:, 0:512], psO1[:], recip[:])
            nc.vector.tensor_scalar_mul(
                osb[:, 512:D], psO2[:, 0 : D - 512], recip[:]
            )
            nc.sync.dma_start(out[qb * P : (qb + 1) * P, :], osb[:])


def _get_nc():
    if "nc" not in _CACHE:
        _CACHE["nc"] = _build_nc()
    return _CACHE["nc"]


def _make_in_maps(matrix, mask, W1_w, W1_b, W2_w, W2_b, v_w):
    matrix = np.asarray(matrix, dtype=np.float32)
    mask = np.asarray(mask, dtype=np.int32)
    W1_w = np.ascontiguousarray(np.asarray(W1_w, dtype=np.float32))
    W2_w = np.ascontiguousarray(np.asarray(W2_w, dtype=np.float32))
    w1b = np.ascontiguousarray(np.asarray(W1_b, dtype=np.float32).reshape(_A, 1))
    w2b = np.ascontiguousarray(np.asarray(W2_b, dtype=np.float32).reshape(_A, 1))
    vw = np.ascontiguousarray(np.asarray(v_w, dtype=np.float32).reshape(_A, 1))

    in_maps = []
    for core in range(_NC):
        b = core // 2
        q0 = (core % 2) * _QPC
        matT = np.ascontiguousarray(matrix[b].T)                       # [D, N]
        matTq = np.ascontiguousarray(matT[:, q0 : q0 + _QPC])          # [D, QPC]
        matv = np.ascontiguousarray(matrix[b])                         # [N, D]
        maskT = np.ascontiguousarray(mask[b, q0 : q0 + _QPC, :, 0].T)  # [N, QPC]
        in_maps.append(
            {
                "matT": matT,
                "matTq": matTq,
                "matv": matv,
                "maskT": maskT,
                "w1w": W1_w,
                "w2w": W2_w,
                "w1b": w1b,
                "w2b": w2b,
                "vw": vw,
            }
        )
    return in_maps


def _run(inputs, trace=False, **kwargs):
    """Run on 8 cores; returns (full_output [B,N,D], BassKernelResults)."""
    from concourse.bass_utils import run_bass_kernel_spmd

    nc = _get_nc()
    in_maps = _make_in_maps(**inputs)
    res = run_bass_kernel_spmd(
        nc, in_maps, core_ids=list(range(_NC)), trace=trace, **kwargs
    )
    output = np.empty((_B, _N, _D), dtype=np.float32)
    for core in range(_NC):
        b = core // 2
        q0 = (core % 2) * _QPC
        output[b, q0 : q0 + _QPC, :] = res.results[core]["out"]
    return output, res


def kernel(**inputs):
    output, _ = _run(inputs, trace=False)
    return output


# revision 5
# speedup vs baseline: 1.0411x; 1.0411x over previous
"""Trainium2 Bass kernel for additive (Bahdanau-style) attention.

Reference computation (per batch b):
    w1 = matrix @ W1_w + W1_b                  # [N, A]
    w2 = matrix @ W2_w + W2_b                  # [N, A]
    scores[i, j] = v . tanh(w1[i] + w2[j])     # [N, N]
    attn = softmax(where(mask, scores, -inf))  # [N, N]
    out = attn @ matrix                        # [N, D]

Shapes: B=4, N=512, D=768, A=128.

Sharding: 8 cores = (batch b = core//2) x (query half = core%2). Each core
owns 256 queries of one batch; all compute is core-local (no collectives).

Per-core dataflow (all "transposed" so no on-chip transposes are needed):
  - w1T [A=128part, 256q], w2T [A, 512k] via PE matmuls from matrix^T.
  - per query q: DVE tensor_scalar broadcast-add  sums = w2T + w1T[:, q]
    (fp32 2x mode), ScalarE tanh on groups of 8 queries (bf16 out).
  - scores^T column: PE matmul lhsT=tanh chunk [A,128k] (bf16, FWL),
    rhs=v [A,1] -> psum column [128k, 1]; 4 key chunks -> S^T in PSUM
    packed [128kpart, (kc, q) free].
  - softmax without max-subtraction (scores are bounded ~|s|<=9):
    exp on ScalarE (PSUM->SBUF, bf16), mask multiply on DVE,
    row sums via PE matmul with an appended ones-column on the AV rhs.
  - AV: out[q, d] = sum_k P^T[k, q] * V[k, d]: lhsT = P^T chunk, rhs = V
    (bf16), accumulated over 4 key chunks; normalization by 1/rowsum fused
    into the PSUM->SBUF copy (per-partition tensor_scalar).
"""

import numpy as np

_B, _N, _D, _A = 4, 512, 768, 128
_NC = 8
_QPC = (_B * _N) // _NC  # 256 queries per core
_P = 128
_KD = _D // _P  # 6 contraction chunks over D
_KC = _N // _P  # 4 key chunks
_QG = 16        # queries per tanh group (ScalarE call)

_CACHE = {}


def _build_nc():
    import concourse.tile as tile
    from concourse import bacc, mybir

    f32 = mybir.dt.float32
    bf16 = mybir.dt.bfloat16
    i32 = mybir.dt.int32

    nc = bacc.Bacc(
        "TRN2",
        target_bir_lowering=False,
        debug=False,
        num_devices=1,
    )

    # Per-core inputs (host does only slicing / transposition).
    matT = nc.dram_tensor("matT", [_D, _N], f32, kind="ExternalInput").ap()
    matTq = nc.dram_tensor("matTq", [_D, _QPC], f32, kind="ExternalInput").ap()
    matv = nc.dram_tensor("matv", [_N, _D], f32, kind="ExternalInput").ap()
    maskT = nc.dram_tensor("maskT", [_N, _QPC], i32, kind="ExternalInput").ap()
    w1w = nc.dram_tensor("w1w", [_D, _A], f32, kind="ExternalInput").ap()
    w2w = nc.dram_tensor("w2w", [_D, _A], f32, kind="ExternalInput").ap()
    w1b = nc.dram_tensor("w1b", [_A, 1], f32, kind="ExternalInput").ap()
    w2b = nc.dram_tensor("w2b", [_A, 1], f32, kind="ExternalInput").ap()
    vw = nc.dram_tensor("vw", [_A, 1], f32, kind="ExternalInput").ap()
    out = nc.dram_tensor("out", [_QPC, _D], f32, kind="ExternalOutput").ap()

    with tile.TileContext(nc) as tc:
        _kernel_body(
            tc, mybir, matT, matTq, matv, maskT, w1w, w2w, w1b, w2b, vw, out
        )
    nc.compile()
    return nc


def _kernel_body(tc, mybir, matT, matTq, matv, maskT, w1w, w2w, w1b, w2b, vw, out):
    nc = tc.nc
    f32 = mybir.dt.float32
    bf16 = mybir.dt.bfloat16
    i32 = mybir.dt.int32
    Tanh = mybir.ActivationFunctionType.Tanh
    Exp = mybir.ActivationFunctionType.Exp
    P, N, D, A, QPC = _P, _N, _D, _A, _QPC
    KD, KC, QG = _KD, _KC, _QG
    NG = P // QG  # tanh groups per 128-query block

    with (
        tc.tile_pool(name="const", bufs=1) as const,
        tc.tile_pool(name="sums", bufs=2) as sums_pool,
        tc.tile_pool(name="tanh", bufs=2) as tanh_pool,
        tc.tile_pool(name="pt", bufs=2) as pt_pool,
        tc.tile_pool(name="osb", bufs=2) as osb_pool,
        tc.tile_pool(name="small", bufs=2) as small_pool,
        tc.tile_pool(name="psS", bufs=2, space="PSUM") as psS_pool,
        tc.tile_pool(name="psO1", bufs=2, space="PSUM") as psO1_pool,
        tc.tile_pool(name="psO2", bufs=2, space="PSUM") as psO2_pool,
    ):
        # ---------------- inputs to SBUF ----------------
        # Projection-critical inputs first (weights, biases, matTq, matT),
        # each D-chunk as its own tile so projections start as chunks land;
        # matv/mask are needed only by the block epilogues and go last.
        w1b_sb = const.tile([P, 1], f32)
        nc.sync.dma_start(w1b_sb[:], w1b)
        w2b_sb = const.tile([P, 1], f32)
        nc.sync.dma_start(w2b_sb[:], w2b)
        vw_sb = const.tile([P, 1], f32)
        nc.sync.dma_start(vw_sb[:], vw)
        v_bf = const.tile([P, 1], bf16)
        nc.vector.tensor_copy(v_bf[:], vw_sb[:])

        w1w_sb = []
        w2w_sb = []
        matTq_sb = []
        matT_sb = []
        for kd in range(KD):
            w1 = const.tile([P, A], f32, tag=f"w1w{kd}")
            nc.sync.dma_start(w1[:], w1w[kd * P : (kd + 1) * P, :])
            w1w_sb.append(w1)
            w2 = const.tile([P, A], f32, tag=f"w2w{kd}")
            nc.sync.dma_start(w2[:], w2w[kd * P : (kd + 1) * P, :])
            w2w_sb.append(w2)
        for kd in range(KD):
            tq = const.tile([P, QPC], f32, tag=f"matTq{kd}")
            nc.sync.dma_start(tq[:], matTq[kd * P : (kd + 1) * P, :])
            matTq_sb.append(tq)
        for kd in range(KD):
            tt = const.tile([P, N], f32, tag=f"matT{kd}")
            nc.sync.dma_start(tt[:], matT[kd * P : (kd + 1) * P, :])
            matT_sb.append(tt)

        # ---------------- projections: w1T [A, QPC], w2T [A, N] ----------------
        ps_w1 = psS_pool.tile([P, N], f32, tag="psS")
        for kd in range(KD):
            nc.tensor.matmul(
                ps_w1[:, :QPC],
                lhsT=w1w_sb[kd][:],
                rhs=matTq_sb[kd][:],
                start=(kd == 0),
                stop=(kd == KD - 1),
            )
        w1T_sb = const.tile([P, QPC], f32)
        nc.vector.tensor_scalar_add(w1T_sb[:], ps_w1[:, :QPC], w1b_sb[:])

        ps_w2 = psS_pool.tile([P, N], f32, tag="psS")
        for kd in range(KD):
            nc.tensor.matmul(
                ps_w2[:],
                lhsT=w2w_sb[kd][:],
                rhs=matT_sb[kd][:],
                start=(kd == 0),
                stop=(kd == KD - 1),
            )
        # w2T in bf16: lets the per-query broadcast-add run in DVE 4x mode
        w2T_bf = const.tile([P, N], bf16)
        nc.vector.tensor_scalar_add(w2T_bf[:], ps_w2[:], w2b_sb[:])

        # ---------------- late inputs (needed only by epilogues) ----------------
        matv_sb = const.tile([P, KC, D], f32)
        nc.sync.dma_start(matv_sb[:], matv.rearrange("(o p) d -> p o d", p=P))
        mask_sb = const.tile([P, KC, QPC], i32)
        nc.sync.dma_start(mask_sb[:], maskT.rearrange("(o p) q -> p o q", p=P))
        mask_bf = const.tile([P, KC, QPC], bf16)
        nc.vector.tensor_copy(mask_bf[:], mask_sb[:])
        # AV rhs with an appended ones column (gives row-sums for free):
        # [:, kc, 0:768] = V chunk, [:, kc, 768] = 1.0
        mov_bf = const.tile([P, KC, D + 2], bf16)
        nc.vector.tensor_copy(mov_bf[:, :, 0:D], matv_sb[:])
        nc.vector.memset(mov_bf[:, :, D : D + 2], 1.0)

        # ---------------- main loop over 128-query blocks ----------------
        for qb in range(QPC // P):
            # scores^T for this block, packed [128 key-part, (kc, q) free]
            psS = psS_pool.tile([P, N], f32, tag="psS")
            for g in range(NG):
                sums = sums_pool.tile([P, QG, N], bf16)
                for j in range(QG):
                    q = qb * P + g * QG + j
                    nc.vector.tensor_scalar_add(
                        sums[:, j, :], w2T_bf[:], w1T_sb[:, q : q + 1]
                    )
                th = tanh_pool.tile([P, QG, N], bf16)
                nc.scalar.activation(th[:], sums[:], Tanh)
                for j in range(QG):
                    ql = g * QG + j  # query index within block (0..127)
                    for kc in range(KC):
                        nc.tensor.matmul(
                            psS[:, kc * P + ql : kc * P + ql + 1],
                            lhsT=th[:, j, kc * P : (kc + 1) * P],
                            rhs=v_bf[:],
                            start=True,
                            stop=True,
                        )

            # exp (no max subtraction needed: |scores| <= sum|v| ~ 9)
            pt = pt_pool.tile([P, N], bf16)
            nc.scalar.activation(pt[:], psS[:], Exp)
            # mask: P^T *= mask^T  (bf16 2x)
            for kc in range(KC):
                nc.vector.tensor_mul(
                    pt[:, kc * P : (kc + 1) * P],
                    pt[:, kc * P : (kc + 1) * P],
                    mask_bf[:, kc, qb * P : (qb + 1) * P],
                )

            # AV + rowsum: out[q, d] = sum_kc P^T[kc].T @ [V | 1]
            psO1 = psO1_pool.tile([P, 512], f32, tag="psO1")
            psO2 = psO2_pool.tile([P, D - 512 + 2], f32, tag="psO2")
            for kc in range(KC):
                lhsT = pt[:, kc * P : (kc + 1) * P]
                nc.tensor.matmul(
                    psO1[:],
                    lhsT=lhsT,
                    rhs=mov_bf[:, kc, 0:512],
                    start=(kc == 0),
                    stop=(kc == KC - 1),
                )
                nc.tensor.matmul(
                    psO2[:],
                    lhsT=lhsT,
                    rhs=mov_bf[:, kc, 512 : D + 2],
                    start=(kc == 0),
                    stop=(kc == KC - 1),
                )

            recip = small_pool.tile([P, 1], f32)
            nc.vector.reciprocal(recip[:], psO2[:, D - 512 : D - 512 + 1])

            osb = osb_pool.tile([P, D], f32)
            nc.vector.tensor_scalar_mul(osb[:, 0:512], psO1[:], recip[:])
            nc.vector.tensor_scalar_mul(
                osb[:, 512:D], psO2[:, 0 : D - 512], recip[:]
            )
            nc.sync.dma_start(out[qb * P : (qb + 1) * P, :], osb[:])


def _get_nc():
    if "nc" not in _CACHE:
        _CACHE["nc"] = _build_nc()
    return _CACHE["nc"]


def _make_in_maps(matrix, mask, W1_w, W1_b, W2_w, W2_b, v_w):
    matrix = np.asarray(matrix, dtype=np.float32)
    mask = np.asarray(mask, dtype=np.int32)
    W1_w = np.ascontiguousarray(np.asarray(W1_w, dtype=np.float32))
    W2_w = np.ascontiguousarray(np.asarray(W2_w, dtype=np.float32))
    w1b = np.ascontiguousarray(np.asarray(W1_b, dtype=np.float32).reshape(_A, 1))
    w2b = np.ascontiguousarray(np.asarray(W2_b, dtype=np.float32).reshape(_A, 1))
    vw = np.ascontiguousarray(np.asarray(v_w, dtype=np.float32).reshape(_A, 1))

    in_maps = []
    for core in range(_NC):
        b = core // 2
        q0 = (core % 2) * _QPC
        matT = np.ascontiguousarray(matrix[b].T)                       # [D, N]
        matTq = np.ascontiguousarray(matT[:, q0 : q0 + _QPC])          # [D, QPC]
        matv = np.ascontiguousarray(matrix[b])                         # [N, D]
        maskT = np.ascontiguousarray(mask[b, q0 : q0 + _QPC, :, 0].T)  # [N, QPC]
        in_maps.append(
            {
                "matT": matT,
                "matTq": matTq,
                "matv": matv,
                "maskT": maskT,
                "w1w": W1_w,
                "w2w": W2_w,
                "w1b": w1b,
                "w2b": w2b,
                "vw": vw,
            }
        )
    return in_maps


def _run(inputs, trace=False, **kwargs):
    """Run on 8 cores; returns (full_output [B,N,D], BassKernelResults)."""
    from concourse.bass_utils import run_bass_kernel_spmd

    nc = _get_nc()
    in_maps = _make_in_maps(**inputs)
    res = run_bass_kernel_spmd(
        nc, in_maps, core_ids=list(range(_NC)), trace=trace, **kwargs
    )
    output = np.empty((_B, _N, _D), dtype=np.float32)
    for core in range(_NC):
        b = core // 2
        q0 = (core % 2) * _QPC
        output[b, q0 : q0 + _QPC, :] = res.results[core]["out"]
    return output, res


def kernel(**inputs):
    output, _ = _run(inputs, trace=False)
    return output


# revision 9
# speedup vs baseline: 1.0876x; 1.0446x over previous
"""Trainium2 Bass kernel for additive (Bahdanau-style) attention.

Reference computation (per batch b):
    w1 = matrix @ W1_w + W1_b                  # [N, A]
    w2 = matrix @ W2_w + W2_b                  # [N, A]
    scores[i, j] = v . tanh(w1[i] + w2[j])     # [N, N]
    attn = softmax(where(mask, scores, -inf))  # [N, N]
    out = attn @ matrix                        # [N, D]

Shapes: B=4, N=512, D=768, A=128.

Sharding: 8 cores = (batch b = core//2) x (query half = core%2). Each core
owns 256 queries of one batch; all compute is core-local (no collectives).

Per-core dataflow (all "transposed" so no on-chip transposes are needed):
  - w1T [A=128part, 256q], w2T [A, 512k] via PE matmuls from matrix^T.
  - per query q: DVE tensor_scalar broadcast-add  sums = w2T + w1T[:, q]
    (fp32 2x mode), ScalarE tanh on groups of 8 queries (bf16 out).
  - scores^T column: PE matmul lhsT=tanh chunk [A,128k] (bf16, FWL),
    rhs=v [A,1] -> psum column [128k, 1]; 4 key chunks -> S^T in PSUM
    packed [128kpart, (kc, q) free].
  - softmax without max-subtraction (scores are bounded ~|s|<=9):
    exp on ScalarE (PSUM->SBUF, bf16), mask multiply on DVE,
    row sums via PE matmul with an appended ones-column on the AV rhs.
  - AV: out[q, d] = sum_k P^T[k, q] * V[k, d]: lhsT = P^T chunk, rhs = V
    (bf16), accumulated over 4 key chunks; normalization by 1/rowsum fused
    into the PSUM->SBUF copy (per-partition tensor_scalar).
"""

import numpy as np

_B, _N, _D, _A = 4, 512, 768, 128
_NC = 8
_QPC = (_B * _N) // _NC  # 256 queries per core
_P = 128
_KD = _D // _P  # 6 contraction chunks over D
_KC = _N // _P  # 4 key chunks
_QG = 16        # queries per tanh group (ScalarE call)

_CACHE = {}


def _build_nc():
    import concourse.tile as tile
    from concourse import bacc, mybir

    f32 = mybir.dt.float32
    bf16 = mybir.dt.bfloat16
    i32 = mybir.dt.int32

    nc = bacc.Bacc(
        "TRN2",
        target_bir_lowering=False,
        debug=False,
        num_devices=1,
    )

    # Per-core inputs (host does only slicing / transposition / layout).
    # All big tensors arrive pre-flattened to [128, W] so each is one
    # contiguous 128-descriptor DMA (DIRECT2D issue cost is per row).
    matT = nc.dram_tensor("matT", [_P, _KD * _N], f32, kind="ExternalInput").ap()
    matTq = nc.dram_tensor("matTq", [_P, _KD * _QPC], f32, kind="ExternalInput").ap()
    matv = nc.dram_tensor("matv", [_P, _KC * _D], f32, kind="ExternalInput").ap()
    maskT = nc.dram_tensor("maskT", [_P, _KC * _QPC], i32, kind="ExternalInput").ap()
    w1w = nc.dram_tensor("w1w", [_D, _A], f32, kind="ExternalInput").ap()
    w2w = nc.dram_tensor("w2w", [_D, _A], f32, kind="ExternalInput").ap()
    w1b = nc.dram_tensor("w1b", [_A, 1], f32, kind="ExternalInput").ap()
    w2b = nc.dram_tensor("w2b", [_A, 1], f32, kind="ExternalInput").ap()
    vw = nc.dram_tensor("vw", [_A, 1], f32, kind="ExternalInput").ap()
    out = nc.dram_tensor("out", [_QPC, _D], f32, kind="ExternalOutput").ap()

    with tile.TileContext(nc) as tc:
        _kernel_body(
            tc, mybir, matT, matTq, matv, maskT, w1w, w2w, w1b, w2b, vw, out
        )
    nc.compile()
    return nc


def _kernel_body(tc, mybir, matT, matTq, matv, maskT, w1w, w2w, w1b, w2b, vw, out):
    nc = tc.nc
    f32 = mybir.dt.float32
    bf16 = mybir.dt.bfloat16
    i32 = mybir.dt.int32
    Tanh = mybir.ActivationFunctionType.Tanh
    Exp = mybir.ActivationFunctionType.Exp
    P, N, D, A, QPC = _P, _N, _D, _A, _QPC
    KD, KC, QG = _KD, _KC, _QG
    NG = P // QG  # tanh groups per 128-query block

    with (
        tc.tile_pool(name="const", bufs=1) as const,
        tc.tile_pool(name="sums", bufs=2) as sums_pool,
        tc.tile_pool(name="tanh", bufs=2) as tanh_pool,
        tc.tile_pool(name="pt", bufs=2) as pt_pool,
        tc.tile_pool(name="osb", bufs=2) as osb_pool,
        tc.tile_pool(name="small", bufs=2) as small_pool,
        tc.tile_pool(name="psS", bufs=2, space="PSUM") as psS_pool,
        tc.tile_pool(name="psO1", bufs=2, space="PSUM") as psO1_pool,
        tc.tile_pool(name="psO2", bufs=2, space="PSUM") as psO2_pool,
    ):
        # ---------------- inputs to SBUF ----------------
        # Projection-critical inputs first (weights, biases, matTq, matT),
        # each D-chunk as its own tile so projections start as chunks land;
        # matv/mask are needed only by the block epilogues and go last.
        w1b_sb = const.tile([P, 1], f32)
        nc.sync.dma_start(w1b_sb[:], w1b)
        w2b_sb = const.tile([P, 1], f32)
        nc.sync.dma_start(w2b_sb[:], w2b)
        vw_sb = const.tile([P, 1], f32)
        nc.sync.dma_start(vw_sb[:], vw)
        v_bf = const.tile([P, 1], bf16)
        nc.vector.tensor_copy(v_bf[:], vw_sb[:])

        w1w_sb = const.tile([P, KD, A], f32)
        nc.sync.dma_start(w1w_sb[:], w1w.rearrange("p (o a) -> p o a", a=A))
        matTq_sb = const.tile([P, KD, QPC], f32)
        nc.sync.dma_start(matTq_sb[:], matTq.rearrange("p (o n) -> p o n", n=QPC))
        w2w_sb = const.tile([P, KD, A], f32)
        nc.sync.dma_start(w2w_sb[:], w2w.rearrange("p (o a) -> p o a", a=A))
        matT_sb = const.tile([P, KD, N], f32)
        nc.sync.dma_start(matT_sb[:], matT.rearrange("p (o n) -> p o n", n=N))

        # ---------------- projections: w1T [A, QPC], w2T [A, N] ----------------
        ps_w1 = psS_pool.tile([P, N], f32, tag="psS")
        for kd in range(KD):
            nc.tensor.matmul(
                ps_w1[:, :QPC],
                lhsT=w1w_sb[:, kd, :],
                rhs=matTq_sb[:, kd, :],
                start=(kd == 0),
                stop=(kd == KD - 1),
            )
        w1T_sb = const.tile([P, QPC], f32)
        nc.vector.tensor_scalar_add(w1T_sb[:], ps_w1[:, :QPC], w1b_sb[:])

        ps_w2 = psS_pool.tile([P, N], f32, tag="psS")
        for kd in range(KD):
            nc.tensor.matmul(
                ps_w2[:],
                lhsT=w2w_sb[:, kd, :],
                rhs=matT_sb[:, kd, :],
                start=(kd == 0),
                stop=(kd == KD - 1),
            )
        # w2T in bf16: lets the per-query broadcast-add run in DVE 2x mode
        w2T_bf = const.tile([P, N], bf16)
        nc.vector.tensor_scalar_add(w2T_bf[:], ps_w2[:], w2b_sb[:])

        # ---------------- late inputs (needed only by epilogues) ----------------
        matv_sb = const.tile([P, KC, D], f32)
        nc.sync.dma_start(matv_sb[:], matv.rearrange("p (o d) -> p o d", d=D))
        mask_sb = const.tile([P, KC, QPC], i32)
        nc.sync.dma_start(mask_sb[:], maskT.rearrange("p (o q) -> p o q", q=QPC))
        mask_bf = const.tile([P, KC, QPC], bf16)
        nc.vector.tensor_copy(mask_bf[:], mask_sb[:])
        # AV rhs with an appended ones column (gives row-sums for free):
        # [:, kc, 0:768] = V chunk, [:, kc, 768] = 1.0
        mov_bf = const.tile([P, KC, D + 2], bf16)
        nc.vector.tensor_copy(mov_bf[:, :, 0:D], matv_sb[:])
        nc.vector.memset(mov_bf[:, :, D : D + 2], 1.0)

        # ---------------- main loop over 128-query blocks ----------------
        for qb in range(QPC // P):
            # scores^T for this block, packed [128 key-part, (kc, q) free]
            psS = psS_pool.tile([P, N], f32, tag="psS")
            # Ramp group sizes at the very start so the first tanh fires as
            # soon as a couple of sums are ready (shorter pipeline fill).
            if qb == 0:
                sizes = [2, 2, 4, 8] + [QG] * ((P - 16) // QG)
            else:
                sizes = [QG] * (P // QG)
            qoff = 0
            for s in sizes:
                sums = sums_pool.tile([P, QG, N], bf16, tag="sums")
                for j in range(s):
                    q = qb * P + qoff + j
                    nc.vector.tensor_scalar_add(
                        sums[:, j, :], w2T_bf[:], w1T_sb[:, q : q + 1]
                    )
                th = tanh_pool.tile([P, QG, N], bf16, tag="tanh")
                nc.scalar.activation(th[:, :s, :], sums[:, :s, :], Tanh)
                for j in range(s):
                    ql = qoff + j  # query index within block (0..127)
                    for kc in range(KC):
                        nc.tensor.matmul(
                            psS[:, kc * P + ql : kc * P + ql + 1],
                            lhsT=th[:, j, kc * P : (kc + 1) * P],
                            rhs=v_bf[:],
                            start=True,
                            stop=True,
                        )
                qoff += s

            # exp (no max subtraction needed: |scores| <= sum|v| ~ 9)
            pt = pt_pool.tile([P, N], bf16)
            nc.scalar.activation(pt[:], psS[:], Exp)
            # mask: P^T *= mask^T  (bf16 2x)
            for kc in range(KC):
                nc.vector.tensor_mul(
                    pt[:, kc * P : (kc + 1) * P],
                    pt[:, kc * P : (kc + 1) * P],
                    mask_bf[:, kc, qb * P : (qb + 1) * P],
                )

            # AV + rowsum: out[q, d] = sum_kc P^T[kc].T @ [V | 1]
            psO1 = psO1_pool.tile([P, 512], f32, tag="psO1")
            psO2 = psO2_pool.tile([P, D - 512 + 2], f32, tag="psO2")
            for kc in range(KC):
                lhsT = pt[:, kc * P : (kc + 1) * P]
                nc.tensor.matmul(
                    psO1[:],
                    lhsT=lhsT,
                    rhs=mov_bf[:, kc, 0:512],
                    start=(kc == 0),
                    stop=(kc == KC - 1),
                )
                nc.tensor.matmul(
                    psO2[:],
                    lhsT=lhsT,
                    rhs=mov_bf[:, kc, 512 : D + 2],
                    start=(kc == 0),
                    stop=(kc == KC - 1),
                )

            recip = small_pool.tile([P, 1], f32)
            nc.vector.reciprocal(recip[:], psO2[:, D - 512 : D - 512 + 1])

            osb = osb_pool.tile([P, D], f32)
            nc.vector.tensor_scalar_mul(osb[:, 0:512], psO1[:], recip[:])
            nc.vector.tensor_scalar_mul(
                osb[:, 512:D], psO2[:, 0 : D - 512], recip[:]
            )
            nc.sync.dma_start(out[qb * P : (qb + 1) * P, :], osb[:])


def _get_nc():
    if "nc" not in _CACHE:
        _CACHE["nc"] = _build_nc()
    return _CACHE["nc"]


def _make_in_maps(matrix, mask, W1_w, W1_b, W2_w, W2_b, v_w):
    matrix = np.asarray(matrix, dtype=np.float32)
    mask = np.asarray(mask, dtype=np.int32)
    W1_w = np.ascontiguousarray(np.asarray(W1_w, dtype=np.float32))
    W2_w = np.ascontiguousarray(np.asarray(W2_w, dtype=np.float32))
    w1b = np.ascontiguousarray(np.asarray(W1_b, dtype=np.float32).reshape(_A, 1))
    w2b = np.ascontiguousarray(np.asarray(W2_b, dtype=np.float32).reshape(_A, 1))
    vw = np.ascontiguousarray(np.asarray(v_w, dtype=np.float32).reshape(_A, 1))

    def flat128(x):
        # [(o*128), W] -> [128, o*W]: chunk-major per partition row
        o = x.shape[0] // _P
        return np.ascontiguousarray(
            x.reshape(o, _P, x.shape[1]).transpose(1, 0, 2).reshape(_P, -1)
        )

    w1w_f = flat128(W1_w)
    w2w_f = flat128(W2_w)

    in_maps = []
    for core in range(_NC):
        b = core // 2
        q0 = (core % 2) * _QPC
        matT = matrix[b].T                              # [D, N]
        matTq = matT[:, q0 : q0 + _QPC]                 # [D, QPC]
        matv = matrix[b]                                # [N, D]
        maskT = mask[b, q0 : q0 + _QPC, :, 0].T         # [N, QPC]
        in_maps.append(
            {
                "matT": flat128(matT),
                "matTq": flat128(matTq),
                "matv": flat128(matv),
                "maskT": flat128(maskT),
                "w1w": w1w_f,
                "w2w": w2w_f,
                "w1b": w1b,
                "w2b": w2b,
                "vw": vw,
            }
        )
    return in_maps


def _run(inputs, trace=False, **kwargs):
    """Run on 8 cores; returns (full_output [B,N,D], BassKernelResults)."""
    from concourse.bass_utils import run_bass_kernel_spmd

    nc = _get_nc()
    in_maps = _make_in_maps(**inputs)
    res = run_bass_kernel_spmd(
        nc, in_maps, core_ids=list(range(_NC)), trace=trace, **kwargs
    )
    output = np.empty((_B, _N, _D), dtype=np.float32)
    for core in range(_NC):
        b = core // 2
        q0 = (core % 2) * _QPC
        output[b, q0 : q0 + _QPC, :] = res.results[core]["out"]
    return output, res


def kernel(**inputs):
    output, _ = _run(inputs, trace=False)
    return output
